# revision 1
# baseline (speedup 1.0000x reference)
"""Trainium2 Bass kernel for nn_FChCombxValEncoder (HDC n-gram encoder).

Computation: idx = quantize(x) -> signal = signals_weight[idx] -> bind with
feat_weight -> 4-gram product with per-step D-rolls -> bundle sum -> sign.

Distribution: feature axis (4096) sharded across 8 cores (512 n-gram starts
each); per-core bundle partials are AllReduced.

Layout trick: each partition p holds FOUR consecutive rows t = 4p+j as four
streams side by side in the free dim (stream pitch 2005, odd, so the +1
D-shift lands 4B-aligned where possible). Then
    U_j = S_j . S_{j+1}(+1)     three streams in-partition; j=3 uses A' =
                                S_0 of partition p+1 via a small split
                                SBUF->SBUF DMA (row 127 pre-zeroed)
    Q_j = U_j . U_{j+2}(+2)     j=0,1 in-partition; j=2,3 use V = U_{0,1}
                                of partition p+1 via a shift-matmul on PE
                                (zero-column => K=128, exact for +/-1)
    R   = ones^T @ Q            PSUM accumulate over the 4 streams + the
                                orphan rows, 512-col segs
so there are no per-row shift matmul/copy chains at all.  The 3 n-grams per
core that need rows beyond the 512-row pack ("orphans", t = 509..511) are
computed on the otherwise-idle GpSimd engine at chunk width and folded into
the same PSUM accumulators.

All values are +/-1 so bf16 is exact; bundle partials are integers < 2^24 so
fp32 PSUM/AllReduce is exact; the output sign never sees zero (4093 odd
terms).  Index quantization is bit-exact via a host fp32 threshold table:
idx = #{k : x >= b_k}.
"""
import sys

sys.path.insert(0, "/opt/trn_rl_repo")

import numpy as np
import ml_dtypes

import concourse.bass as bass
import concourse.bacc as bacc
import concourse.tile as tile
import concourse.mybir as mybir
from concourse.bass_utils import run_bass_kernel_spmd

# ---- problem constants ----
MAX_VAL = 52000.0
MIN_VAL = -53000.0
NUM_LEVELS = 1000
NGRAM = 4
D = 10000
NFEAT = 4096
NCORE = 8

PER_CORE = 512
ROLL = NGRAM - 1

W = 2000                      # D-chunk width
NCHUNK = D // W               # 5
PS = 2005                     # packed stream pitch (odd), stream data width PS
UP = 2004                     # U stream pitch (even), width 2002
QP = 2000                     # Q stream pitch
DPAD = D + PS - W             # table/orphan row width 10005 (max e0 + PS)
NORPH = 6                     # orphan rows per core (t = 509..514)
SEGS = [(0, 512), (512, 1024), (1024, 1536), (1536, 2000)]

F32 = mybir.dt.float32
BF16 = mybir.dt.bfloat16
I32 = mybir.dt.int32
_BF = ml_dtypes.bfloat16


# ---------------------------------------------------------------- host prep
def _f2o(u):
    b = u.view(np.uint32).astype(np.int64)
    return np.where(b < 0x80000000, b + 0x80000000, 0xFFFFFFFF - b)


def _o2f(o):
    b = np.where(o >= 0x80000000, o - 0x80000000, 0xFFFFFFFF - o).astype(np.uint64)
    return b.astype(np.uint32).view(np.float32)


def _g(v):
    v = v.astype(np.float32)
    t = (v - np.float32(MIN_VAL)).astype(np.float32)
    t = (t / np.float32(MAX_VAL - MIN_VAL)).astype(np.float32)
    t = (t * np.float32(NUM_LEVELS - 1)).astype(np.float32)
    return np.clip(np.round(t), 0.0, float(NUM_LEVELS - 1))


def _thresholds():
    ks = np.arange(1, NUM_LEVELS, dtype=np.float32)
    lo = _f2o(np.full(ks.shape, np.float32(MIN_VAL) - np.float32(2.0)))
    hi = _f2o(np.full(ks.shape, np.float32(MAX_VAL) + np.float32(2.0)))
    for _ in range(64):
        mid = (lo + hi) // 2
        ge = _g(_o2f(mid)) >= ks
        hi = np.where(ge, mid, hi)
        lo = np.where(ge, lo, mid)
        if np.all(hi - lo <= 1):
            break
    return _o2f(hi)


_CACHE = {}


def _host_constants():
    if "thr" not in _CACHE:
        _CACHE["thr"] = np.tile(_thresholds()[None, :], (128, 1)).astype(np.float32)
        sh1 = np.zeros((128, 128), dtype=_BF)
        for i in range(127):
            sh1[i + 1, i] = 1.0          # V[m] = U[m+1]; col 127 zero
        _CACHE["sh1"] = sh1
        _CACHE["ones_red"] = np.ones((128, 1), dtype=_BF)
        _CACHE["zrow"] = np.zeros((1, PS), dtype=_BF)
    return _CACHE


# ---------------------------------------------------------------- program
def _build_program():
    nc = bacc.Bacc("TRN2", target_bir_lowering=False, debug=False,
                   num_devices=NCORE)

    x4_d = nc.dram_tensor("x4", (128, 4), F32, kind="ExternalInput")
    xo_d = nc.dram_tensor("x_orph", (NORPH, 1), F32, kind="ExternalInput")
    thr_d = nc.dram_tensor("thr", (128, NUM_LEVELS - 1), F32, kind="ExternalInput")
    table_d = nc.dram_tensor("table", (NUM_LEVELS, DPAD), BF16, kind="ExternalInput")
    feat4_d = nc.dram_tensor("feat4", (NCHUNK, 128, 4 * PS), BF16, kind="ExternalInput")
    feato_d = nc.dram_tensor("feat_orph", (NCHUNK, NORPH, PS), BF16, kind="ExternalInput")
    sh1_d = nc.dram_tensor("sh1", (128, 128), BF16, kind="ExternalInput")
    onr_d = nc.dram_tensor("ones_red", (128, 1), BF16, kind="ExternalInput")
    zrow_d = nc.dram_tensor("zrow", (1, PS), BF16, kind="ExternalInput")
    out_d = nc.dram_tensor("out", (1, D), F32, kind="ExternalOutput")

    cc_in = nc.dram_tensor("cc_in", (1, D), F32)
    cc_out = nc.dram_tensor("cc_out", (1, D), F32, addr_space="Shared")

    NTH = NUM_LEVELS - 1
    WU = W + 2                              # U stream data width (2002)

    # raw double-buffered A' tensors; row 127 kept zero
    a1_raw = [nc.alloc_sbuf_tensor(f"a1f{i}", [128, WU], BF16).ap()
              for i in range(2)]

    with tile.TileContext(nc) as tc:
        with tc.tile_pool(name="const", bufs=1) as cpool, \
             tc.tile_pool(name="loads", bufs=2) as lpool, \
             tc.tile_pool(name="work", bufs=2) as wpool, \
             tc.tile_pool(name="orph", bufs=1) as opool, \
             tc.tile_pool(name="vone", bufs=1) as vpool, \
             tc.tile_pool(name="pseg", bufs=4, space="PSUM") as pseg, \
             tc.tile_pool(name="pacc", bufs=1, space="PSUM") as pacc:

            sh1 = cpool.tile([128, 128], BF16)
            nc.sync.dma_start(out=sh1[:, :], in_=sh1_d[:, :])
            onr = cpool.tile([128, 1], BF16)
            nc.sync.dma_start(out=onr[:, :], in_=onr_d[:, :])
            thr = cpool.tile([128, NTH], F32)
            nc.sync.dma_start(out=thr[:, :], in_=thr_d[:, :])
            x4 = cpool.tile([128, 4], F32)
            nc.sync.dma_start(out=x4[:, :], in_=x4_d[:, :])
            xo = cpool.tile([NORPH, 1], F32)
            nc.sync.dma_start(out=xo[:, :], in_=xo_d[:, :])

            for a1 in a1_raw:
                nc.sync.dma_start(out=a1[127:128, 0:PS - 3], in_=zrow_d[0:1, 0:PS - 3])

            # ---- per-stream indices ----
            idx_tiles = []
            for j in range(4):
                ge = opool.tile([128, NTH], F32, tag="ge")
                nc.vector.tensor_scalar(
                    out=ge[:, :], in0=thr[:, :], scalar1=x4[:, j:j + 1],
                    scalar2=None, op0=mybir.AluOpType.is_le)
                idxf = opool.tile([128, 1], F32, tag="idxf")
                nc.vector.tensor_reduce(out=idxf[:, :], in_=ge[:, :],
                                        axis=mybir.AxisListType.X,
                                        op=mybir.AluOpType.add)
                it = cpool.tile([128, 1], I32, tag=f"idx{j}")
                nc.vector.tensor_copy(out=it[:, :], in_=idxf[:, :])
                idx_tiles.append(it)
            geo = opool.tile([NORPH, NTH], F32, tag="geo")
            nc.vector.tensor_scalar(
                out=geo[:, :], in0=thr[0:NORPH, :], scalar1=xo[:, 0:1],
                scalar2=None, op0=mybir.AluOpType.is_le)
            idxfo = opool.tile([NORPH, 1], F32, tag="idxfo")
            nc.vector.tensor_reduce(out=idxfo[:, :], in_=geo[:, :],
                                    axis=mybir.AxisListType.X,
                                    op=mybir.AluOpType.add)
            idxo = cpool.tile([NORPH, 1], I32, tag="idxo")
            nc.vector.tensor_copy(out=idxo[:, :], in_=idxfo[:, :])

            for c in range(NCHUNK):
                e0 = c * W

                # ---------- packed loads ----------
                sig4 = lpool.tile([128, 4 * PS], BF16, tag="sig4")
                for j in range(4):
                    nc.gpsimd.indirect_dma_start(
                        out=sig4[:, j * PS:(j + 1) * PS], out_offset=None,
                        in_=table_d[:, :],
                        in_offset=bass.IndirectOffsetOnAxis(
                            ap=idx_tiles[j][:, 0:1], axis=0),
                        element_offset=e0,
                    )
                fe4 = lpool.tile([128, 4 * PS], BF16, tag="fe4")
                nc.sync.dma_start(out=fe4[:, :], in_=feat4_d[c, :, :])

                # ---------- orphan loads ----------
                sig_o = opool.tile([NORPH, PS], BF16, tag="sig_o")
                nc.gpsimd.indirect_dma_start(
                    out=sig_o[:, :], out_offset=None,
                    in_=table_d[:, :],
                    in_offset=bass.IndirectOffsetOnAxis(ap=idxo[:, 0:1], axis=0),
                    element_offset=e0,
                )
                fe_o = opool.tile([NORPH, PS], BF16, tag="fe_o")
                nc.sync.dma_start(out=fe_o[:, :], in_=feato_d[c, :, :])

                # ---------- packed S (in place) ----------
                nc.vector.tensor_tensor(out=sig4[:, :], in0=sig4[:, :],
                                        in1=fe4[:, :], op=mybir.AluOpType.mult)

                # A'[p] = S_0[p+1, 1:2003]  (split SBUF->SBUF DMA)
                a1 = a1_raw[c % 2]
                qs = [nc.sync, nc.scalar, nc.gpsimd]
                for k in range(8):
                    n = 16 if k < 7 else 15
                    qs[k % 3].dma_start(
                        out=a1[16 * k:16 * k + n, :],
                        in_=sig4[16 * k + 1:16 * k + 1 + n, 1:1 + WU])

                # ---------- orphan S / U / Q on gpsimd ----------
                nc.vector.tensor_tensor(out=sig_o[:, :], in0=sig_o[:, :],
                                        in1=fe_o[:, :], op=mybir.AluOpType.mult)
                s1_o = opool.tile([NORPH, PS], BF16, tag="s1_o")
                nc.scalar.dma_start(out=s1_o[0:NORPH - 1, 0:PS - 1],
                                    in_=sig_o[1:NORPH, 1:PS])
                u_o = opool.tile([NORPH, WU], BF16, tag="u_o")
                nc.vector.tensor_tensor(out=u_o[0:NORPH - 1, :],
                                        in0=sig_o[0:NORPH - 1, 0:WU],
                                        in1=s1_o[0:NORPH - 1, 0:WU],
                                        op=mybir.AluOpType.mult)
                u1_o = opool.tile([NORPH, W], BF16, tag="u1_o")
                nc.scalar.dma_start(out=u1_o[0:NORPH - 3, :],
                                    in_=u_o[2:NORPH - 1, 2:2 + W])
                q_o = opool.tile([NORPH, W], BF16, tag="q_o")
                nc.vector.tensor_tensor(out=q_o[0:NORPH - 3, :],
                                        in0=u_o[0:NORPH - 3, 0:W],
                                        in1=u1_o[0:NORPH - 3, :],
                                        op=mybir.AluOpType.mult)

                # ---------- packed U ----------
                u4 = wpool.tile([128, 4 * UP], BF16, tag="u4")
                for j in range(3):
                    nc.vector.tensor_tensor(
                        out=u4[:, j * UP:j * UP + WU],
                        in0=sig4[:, j * PS:j * PS + WU],
                        in1=sig4[:, (j + 1) * PS + 1:(j + 1) * PS + 1 + WU],
                        op=mybir.AluOpType.mult)
                nc.vector.tensor_tensor(
                    out=u4[:, 3 * UP:3 * UP + WU],
                    in0=sig4[:, 3 * PS:3 * PS + WU],
                    in1=a1[:, :],
                    op=mybir.AluOpType.mult)

                # ---------- V = U_{0,1}[p+1, +2] via shift matmul ----------
                v4 = vpool.tile([128, 2 * W], BF16, tag="v4")
                for s in range(2):
                    for a0, a1s in SEGS:
                        vp = pseg.tile([128, 512], F32, tag="v")
                        nc.tensor.matmul(
                            out=vp[:, 0:a1s - a0],
                            lhsT=sh1[:, :],
                            rhs=u4[:, s * UP + 2 + a0:s * UP + 2 + a1s],
                            start=True, stop=True)
                        nc.scalar.copy(out=v4[:, s * W + a0:s * W + a1s],
                                       in_=vp[:, 0:a1s - a0])

                # ---------- packed Q ----------
                q4 = wpool.tile([128, 4 * QP], BF16, tag="q4")
                for j in range(2):
                    nc.vector.tensor_tensor(
                        out=q4[:, j * QP:(j + 1) * QP],
                        in0=u4[:, j * UP:j * UP + W],
                        in1=u4[:, (j + 2) * UP + 2:(j + 2) * UP + 2 + W],
                        op=mybir.AluOpType.mult)
                for j in range(2, 4):
                    nc.vector.tensor_tensor(
                        out=q4[:, j * QP:(j + 1) * QP],
                        in0=u4[:, j * UP:j * UP + W],
                        in1=v4[:, (j - 2) * W:(j - 1) * W],
                        op=mybir.AluOpType.mult)

                # ---------- bundle reduce (packed + orphan) ----------
                accp = pacc.tile([1, W], F32, tag="acc")
                for a0, a1s in SEGS:
                    for j in range(4):
                        nc.tensor.matmul(out=accp[0:1, a0:a1s],
                                         lhsT=onr[:, 0:1],
                                         rhs=q4[:, j * QP + a0:j * QP + a1s],
                                         start=(j == 0), stop=False)
                    nc.tensor.matmul(out=accp[0:1, a0:a1s],
                                     lhsT=onr[0:NORPH - 3, 0:1],
                                     rhs=q_o[0:NORPH - 3, a0:a1s],
                                     start=False, stop=True)
                stg = wpool.tile([1, W], F32, tag="stg")
                nc.scalar.copy(out=stg[:, :], in_=accp[0:1, :])
                nc.sync.dma_start(out=cc_in[0:1, e0:e0 + W], in_=stg[:, :])

            nc.gpsimd.collective_compute(
                "AllReduce", mybir.AluOpType.add,
                ins=[cc_in[:, :]], outs=[cc_out[:, :]],
                replica_groups=[list(range(NCORE))],
            )

            # ---- sign + roll-by-3 output ----
            PR, PW = 125, 80
            r = wpool.tile([PR, PW], F32, tag="fin")
            nc.sync.dma_start(
                out=r[:, :],
                in_=cc_out[:, :].rearrange("o (p w) -> (o p) w", p=PR))
            t1 = wpool.tile([PR, PW], F32, tag="fin2")
            nc.vector.tensor_scalar(out=t1[:, :], in0=r[:, :], scalar1=0.0,
                                    scalar2=2.0, op0=mybir.AluOpType.is_gt,
                                    op1=mybir.AluOpType.mult)
            sg = wpool.tile([PR, PW], F32, tag="fin3")
            nc.vector.tensor_scalar(out=sg[:, :], in0=t1[:, :], scalar1=-1.0,
                                    scalar2=None, op0=mybir.AluOpType.add)
            nfull = (D - ROLL) // PW
            rem = D - ROLL - nfull * PW
            nc.sync.dma_start(out=out_d[0:1, ROLL:ROLL + nfull * PW],
                              in_=sg[0:nfull, :])
            nc.sync.dma_start(out=out_d[0:1, ROLL + nfull * PW:D],
                              in_=sg[nfull:nfull + 1, 0:rem])
            nc.sync.dma_start(out=out_d[0:1, 0:ROLL],
                              in_=sg[nfull:nfull + 1, rem:PW])

    nc.compile()
    return nc


TRACE = False
LAST_RESULT = None


def _pad_rows(fw, base, n):
    """rows [base, base+n) of fw, zero-padded past NFEAT, with DPAD wrap."""
    out = np.zeros((n, DPAD), dtype=_BF)
    nreal = max(0, min(n, NFEAT - base))
    if nreal > 0:
        fb = fw[base:base + nreal].astype(_BF)
        out[:nreal, :D] = fb
        out[:nreal, D:] = fb[:, :DPAD - D]
    return out


def _make_in_maps(xf, sw, fw, consts):
    table = np.empty((NUM_LEVELS, DPAD), dtype=_BF)
    table[:, :D] = sw.astype(_BF)
    table[:, D:] = table[:, :DPAD - D]

    in_maps = []
    for m in range(NCORE):
        base = PER_CORE * m

        # packed feat: feat4[c, p, j*PS + e] = fw_pad[base + 4p + j, c*W + e]
        fp = _pad_rows(fw, base, PER_CORE)              # (512, DPAD)
        fp4 = np.zeros((NCHUNK, 128, 4 * PS), dtype=_BF)
        for c in range(NCHUNK):
            sl = fp[:, c * W:c * W + PS]                # (512, PS)
            fp4[c] = sl.reshape(128, 4 * PS)

        # orphan feat rows base+509 .. base+514
        fo = _pad_rows(fw, base + PER_CORE - 3, NORPH)  # (6, DPAD)
        fo5 = np.zeros((NCHUNK, NORPH, PS), dtype=_BF)
        for c in range(NCHUNK):
            fo5[c] = fo[:, c * W:c * W + PS]

        xr = np.full(PER_CORE + NORPH, xf[-1], dtype=np.float32)
        nreal = min(PER_CORE + 3, NFEAT - base)
        xr[:nreal] = xf[base:base + nreal]
        x4 = xr[:PER_CORE].reshape(128, 4).copy()
        xo = xr[PER_CORE - 3:PER_CORE - 3 + NORPH].reshape(NORPH, 1).copy()

        in_maps.append({
            "x4": x4,
            "x_orph": xo,
            "thr": consts["thr"],
            "table": table,
            "feat4": fp4,
            "feat_orph": fo5,
            "sh1": consts["sh1"],
            "ones_red": consts["ones_red"],
            "zrow": consts["zrow"],
        })
    return in_maps


def kernel(x, signals_weight, feat_weight):
    global LAST_RESULT
    consts = _host_constants()

    if "nc" not in _CACHE:
        _CACHE["nc"] = _build_program()
    nc = _CACHE["nc"]

    xf = np.asarray(x, dtype=np.float32).reshape(-1)
    sw = np.asarray(signals_weight, dtype=np.float32)
    fw = np.asarray(feat_weight, dtype=np.float32)
    in_maps = _make_in_maps(xf, sw, fw, consts)

    res = run_bass_kernel_spmd(nc, in_maps, list(range(NCORE)), trace=TRACE)
    LAST_RESULT = res
    return np.asarray(res.results[0]["out"], dtype=np.float32)

